# revision 18
# baseline (speedup 1.0000x reference)
"""MHSA block (patch-embed conv + relative-pos attention + MLP) on 8 NeuronCores.

Sharding: data-parallel over batch (64 images -> 8 per core), weights
replicated. v2:
 - Phase 1 fuses conv + peLN + geo + LN1 + transpose.
 - QKV phase -> q,k feature-major; v token-major with fused ones-column.
 - Attention: S^T per key-chunk with keys reordered (pixel keys first,
   zero-bias extra token last), one batched exp per head, softmax
   denominator piggybacked as a 65th lhsT column, per-image batched
   reciprocal, 1/Z broadcast via tiny block matmuls.
 - proj+LN2+FC1+FC2 fused over 512-token chunks; y2/h2T/h3 SBUF-resident.
 - DMAs batched; issued on sync+scalar DGE.
"""
import numpy as np
import ml_dtypes
import concourse.bass as bass
import concourse.bacc as bacc
import concourse.tile as tile
from concourse import mybir
from concourse import bass_utils
from concourse.masks import make_identity

BF = ml_dtypes.bfloat16
B, CIN, D, HEADS, HD = 64, 384, 768, 12, 64
GS, ET, N = 16, 1, 257
BL = B // 8              # images per core
NT = BL * N              # 2056 packed tokens per core
MLP = 4 * D
COLT = [(c, min(512, NT - c)) for c in range(0, NT, 512)]

_CACHE = {}
_LAST_MAPS = None


def _rel_bias(rpb_table):
    coords = np.stack(np.meshgrid(np.arange(GS), np.arange(GS), indexing='ij'))
    cf = coords.reshape(2, -1)
    rel = (cf[:, :, None] - cf[:, None, :]).transpose(1, 2, 0)
    rel[:, :, 0] += GS - 1
    rel[:, :, 1] += GS - 1
    rel[:, :, 0] *= 2 * GS - 1
    idx = rel.sum(-1)
    out = np.zeros((N, N), dtype=np.int32)
    out[ET:, ET:] = idx
    bias = rpb_table[out]                    # [N, N, HEADS]
    return bias.transpose(2, 0, 1).astype(np.float32)   # [HEADS, q=N, key=N]


def _ln_pair(tc, nc, pools, xt, ts):
    """mean/rstd of xt[:ts, :768] -> (mean, rstd) [ts,1] f32 tiles."""
    st = pools.tile([128, 3, nc.vector.BN_STATS_DIM], mybir.dt.float32, tag="lnst")
    xg = xt.rearrange("p (n f) -> p n f", f=256)
    for i in range(3):
        nc.vector.bn_stats(out=st[:ts, i], in_=xg[:ts, i])
    mv = pools.tile([128, nc.vector.BN_AGGR_DIM], mybir.dt.float32, tag="lnmv")
    nc.vector.bn_aggr(out=mv[:ts], in_=st[:ts])
    eps = pools.tile([128, 1], mybir.dt.float32, tag="lneps")
    nc.vector.memset(eps, 1e-5)
    rs = pools.tile([128, 1], mybir.dt.float32, tag="lnrs")
    nc.scalar.activation(out=rs[:ts], in_=mv[:ts, 1:2],
                         func=mybir.ActivationFunctionType.Sqrt, bias=eps[:ts])
    nc.vector.reciprocal(out=rs[:ts], in_=rs[:ts])
    return mv, rs


def build():
    nc = bacc.Bacc("TRN2", target_bir_lowering=False, debug=False)
    f32, bf16 = mybir.dt.float32, mybir.dt.bfloat16
    SUB, MUL = mybir.AluOpType.subtract, mybir.AluOpType.mult
    EXP = mybir.ActivationFunctionType.Exp
    di = lambda n, s, d: nc.dram_tensor(n, s, d, kind="ExternalInput").ap()
    x_in = di("x_in", [BL, 3, 128, 32, 32], bf16)
    convw = di("convw", [27, 128, 768], bf16)
    convb_bc = di("convb_bc", [128, 768], f32)
    peg_bc = di("peg_bc", [128, 768], f32)
    geo2 = di("geo2", [2, 128, 768], f32)
    y0row = di("y0row", [1, 768], f32)
    qkw = di("qkw", [6, 128, 1536], bf16)
    qkb_t = di("qkb_t", [128, 12], f32)
    wv_in = di("wv_in", [6, 128, 768], bf16)
    vb_bc = di("vb_bc", [128, 768], f32)
    biasT4 = di("biasT4", [12, 2, 128, 257], bf16)
    eye12 = di("eye12", [12, 12, 64], bf16)
    projw = di("projw", [6, 128, 768], bf16)
    projb_bc = di("projb_bc", [128, 768], f32)
    fc1w = di("fc1w", [6, 128, MLP], bf16)
    fc1b_t = di("fc1b_t", [128, 24], f32)
    fc2w = di("fc2w", [24, 128, 768], bf16)
    fc2b_bc = di("fc2b_bc", [128, 768], f32)
    out_d = nc.dram_tensor("out_d", [NT, 768], f32, kind="ExternalOutput").ap()

    with tile.TileContext(nc) as tc:
        with tc.tile_pool(name="dram", bufs=1, space="DRAM") as dpool:
            y_d = dpool.tile([NT, 768], f32)
            h_d = dpool.tile([NT, 768], bf16)
            qkT_d = dpool.tile([12, 128, NT], bf16)
            va_d = dpool.tile([NT, 12, 65], bf16)
            oT_d = dpool.tile([6, 128, NT], bf16)

            # ---- Phase 1: conv + peLN + geo -> y_d; LN1 + transpose -> hT_d ----
            with tc.tile_pool(name="cw", bufs=1) as cw, \
                 tc.tile_pool(name="cx", bufs=2) as cx, \
                 tc.tile_pool(name="cps", bufs=6, space="PSUM") as cps, \
                 tc.tile_pool(name="cy", bufs=3) as cy:
                wsb = cw.tile([128, 27, 768], bf16)
                nc.sync.dma_start(out=wsb, in_=convw.rearrange("i p f -> p i f"))
                cbc = cw.tile([128, 768], f32)
                nc.sync.dma_start(out=cbc, in_=convb_bc)
                pgc = cw.tile([128, 768], f32)
                nc.sync.dma_start(out=pgc, in_=peg_bc)
                gsb = cw.tile([128, 2, 768], f32)
                nc.sync.dma_start(out=gsb, in_=geo2.rearrange("t p f -> p t f"))
                y0sb = cw.tile([1, 768], f32)
                nc.sync.dma_start(out=y0sb, in_=y0row)
                # token-0 row (identical for every image): y store + LN1 + h_d rows
                for b in range(BL):
                    nc.scalar.dma_start(out=y_d[b * N:b * N + 1, :], in_=y0sb)
                mv0, rs0 = _ln_pair(tc, nc, cy, y0sb, 1)
                hb0 = cy.tile([1, 768], bf16, tag="hb0")
                nc.vector.tensor_scalar(out=hb0, in0=y0sb, scalar1=mv0[:1, 0:1],
                                        scalar2=rs0[:1], op0=SUB, op1=MUL)
                for b in range(BL):
                    nc.scalar.dma_start(out=h_d[b * N:b * N + 1, :], in_=hb0)
                for b in range(BL):
                    xp = cx.tile([128, 3, 1089], bf16, tag="xpad")
                    nc.gpsimd.memset(xp, 0.0)
                    for c in range(3):
                        dst = bass.AP(tensor=xp.tensor, offset=xp.offset + c * 1089 + 34,
                                      ap=[xp.ap[0], [33, 32], [1, 32]])
                        nc.sync.dma_start(out=dst, in_=x_in[b, c])
                    for t in range(2):
                        col = cx.tile([128, 27, 128], bf16, tag="col")
                        for kh in range(3):
                            for kw in range(3):
                                idx = (kh * 3 + kw) * 3
                                src = bass.AP(
                                    tensor=xp.tensor,
                                    offset=xp.offset + (16 * t + kh) * 33 + kw,
                                    ap=[xp.ap[0], [1089, 3], [66, 8], [2, 16]])
                                nc.vector.tensor_copy(
                                    col[:, idx:idx + 3].rearrange("p c (r w) -> p c r w", r=8),
                                    src)
                        yt = cy.tile([128, 768], f32, tag="yt")
                        for nh in range(2):
                            ps = cps.tile([128, 384], f32, tag="cpsum")
                            for i in range(27):
                                nc.tensor.matmul(ps, col[:, i], wsb[:, i, nh * 384:(nh + 1) * 384],
                                                 start=(i == 0), stop=(i == 26))
                            nc.vector.tensor_add(yt[:, nh * 384:(nh + 1) * 384], ps,
                                                 cbc[:, nh * 384:(nh + 1) * 384])
                        mv, rs = _ln_pair(tc, nc, cy, yt, 128)
                        nc.vector.tensor_scalar(out=yt, in0=yt, scalar1=mv[:, 0:1], scalar2=rs,
                                                op0=SUB, op1=MUL)
                        nc.vector.tensor_mul(yt, yt, pgc)
                        nc.vector.tensor_add(yt, yt, gsb[:, t])
                        r0 = b * N + 1 + t * 128
                        nc.scalar.dma_start(out=y_d[r0:r0 + 128, :], in_=yt)
                        mv1, rs1 = _ln_pair(tc, nc, cy, yt, 128)
                        hb = cy.tile([128, 768], bf16, tag="hb")
                        nc.vector.tensor_scalar(out=hb, in0=yt, scalar1=mv1[:, 0:1],
                                                scalar2=rs1, op0=SUB, op1=MUL)
                        nc.sync.dma_start(out=h_d[r0:r0 + 128, :], in_=hb)

            # ---- Phase 3: QKV -> qkT_d (q,k feature-major), va_d (v rows + ones) ----
            with tc.tile_pool(name="qw", bufs=1) as qw, \
                 tc.tile_pool(name="qa", bufs=2) as qa, \
                 tc.tile_pool(name="qs", bufs=2) as qs_pool, \
                 tc.tile_pool(name="qp", bufs=4, space="PSUM") as qp, \
                 tc.tile_pool(name="vp", bufs=4, space="PSUM") as vp:
                wq = qw.tile([128, 6, 1536], bf16)
                nc.sync.dma_start(out=wq, in_=qkw.rearrange("k p f -> p k f"))
                wvs = qw.tile([128, 6, 768], bf16)
                nc.sync.dma_start(out=wvs, in_=wv_in.rearrange("k p f -> p k f"))
                qb = qw.tile([128, 12], f32)
                nc.sync.dma_start(out=qb, in_=qkb_t)
                vbc = qw.tile([128, 768], f32)
                nc.sync.dma_start(out=vbc, in_=vb_bc)
                for (c0, cs) in COLT:
                    ht = qa.tile([128, 6, 512], bf16, tag="qh")
                    for k in range(6):
                        src = h_d[c0:c0 + cs, k * 128:(k + 1) * 128]
                        if cs % 16 == 0:
                            nc.sync.dma_start(out=ht[:, k, :cs], in_=src, transpose=True)
                        else:
                            nc.sync.dma_start(out=ht[:, k, :cs], in_=src.rearrange("a b -> b a"))
                    ev = qs_pool.tile([128, 12, 512], bf16, tag="qev")
                    for dch in range(12):
                        ps = qp.tile([128, 512], f32, tag="qps")
                        for k in range(6):
                            nc.tensor.matmul(ps[:, :cs], wq[:, k, dch * 128:(dch + 1) * 128],
                                             ht[:, k, :cs], start=(k == 0), stop=(k == 5))
                        nc.vector.tensor_scalar_add(out=ev[:, dch, :cs], in0=ps[:, :cs],
                                                    scalar1=qb[:, dch:dch + 1])
                    nc.sync.dma_start(out=qkT_d[:, :, c0:c0 + cs].rearrange("d p t -> p d t"),
                                      in_=ev[:, :, :cs])
                    nsub = (cs + 127) // 128
                    for s in range(nsub):
                        s0 = c0 + s * 128
                        ss = min(128, c0 + cs - s0)
                        vsb = qs_pool.tile([128, 12, 65], bf16, tag="vsb")
                        nc.gpsimd.memset(vsb[:ss], 1.0)
                        for half in range(2):
                            psv = vp.tile([128, 384], f32, tag="vps")
                            for k in range(6):
                                nc.tensor.matmul(psv[:ss], ht[:, k, s * 128:s * 128 + ss],
                                                 wvs[:, k, half * 384:(half + 1) * 384],
                                                 start=(k == 0), stop=(k == 5))
                            nc.vector.tensor_add(
                                vsb[:ss, half * 6:(half + 1) * 6, 0:64],
                                psv[:ss].rearrange("p (h f) -> p h f", f=64),
                                vbc[:ss, half * 384:(half + 1) * 384].rearrange("p (h f) -> p h f", f=64))
                        nc.scalar.dma_start(out=va_d[s0:s0 + ss], in_=vsb[:ss])

            # ---- late weights (prefetch during attention) ----
            with tc.tile_pool(name="fw", bufs=1) as fw:
                wp = fw.tile([128, 6, 768], bf16)
                nc.sync.dma_start(out=wp, in_=projw.rearrange("k p f -> p k f"))
                pbc = fw.tile([128, 768], f32)
                nc.sync.dma_start(out=pbc, in_=projb_bc)
                w1 = fw.tile([128, 6, MLP], bf16)
                nc.sync.dma_start(out=w1, in_=fc1w.rearrange("k p f -> p k f"))
                b1 = fw.tile([128, 24], f32)
                nc.sync.dma_start(out=b1, in_=fc1b_t)
                w2 = fw.tile([128, 24, 768], bf16)
                nc.sync.dma_start(out=w2, in_=fc2w.rearrange("k p f -> p k f"))
                b2c = fw.tile([128, 768], f32)
                nc.sync.dma_start(out=b2c, in_=fc2b_bc)

                # ---- Phase 4: attention -> oT_d ----
                with tc.tile_pool(name="ac", bufs=1) as ac, \
                     tc.tile_pool(name="aq", bufs=2) as aq, \
                     tc.tile_pool(name="aw", bufs=4) as aw, \
                     tc.tile_pool(name="asp", bufs=2, space="PSUM") as asp, \
                     tc.tile_pool(name="as2", bufs=2, space="PSUM") as as2, \
                     tc.tile_pool(name="aop", bufs=2, space="PSUM") as aop, \
                     tc.tile_pool(name="azp", bufs=2, space="PSUM") as azp:
                    bsb = ac.tile([128, 12, 2, 257], bf16)
                    nc.sync.dma_start(out=bsb, in_=biasT4.rearrange("h c p q -> p h c q"))
                    esb = ac.tile([12, 12, 64], bf16)
                    nc.sync.dma_start(out=esb, in_=eye12)
                    for b in range(BL):
                        qt = aq.tile([128, 6, 257], bf16, tag="qt")
                        nc.sync.dma_start(out=qt, in_=qkT_d[0:6, :, b * N:(b + 1) * N].rearrange("g p t -> p g t"))
                        kt = aq.tile([128, 6, 257], bf16, tag="kt")
                        nc.sync.dma_start(out=kt, in_=qkT_d[6:12, :, b * N:(b + 1) * N].rearrange("g p t -> p g t"))
                        va = aq.tile([128, 3, 12, 65], bf16, tag="va")
                        nc.scalar.dma_start(out=va[:, 0], in_=va_d[b * N + 1:b * N + 129])
                        nc.scalar.dma_start(out=va[:, 1], in_=va_d[b * N + 129:b * N + 257])
                        nc.scalar.dma_start(out=va[:1, 2], in_=va_d[b * N:b * N + 1])
                        oc65 = aq.tile([65, 12, 257], f32, tag="oc65")
                        zsb = aq.tile([12, 257], f32, tag="zsb")
                        for h in range(12):
                            g, po = h // 2, (h % 2) * 64
                            ssb2 = aw.tile([128, 2, 257], bf16, tag="ssb2")
                            expst = aw.tile([128, 2, 257], bf16, tag="expst")
                            for ci in range(2):
                                sp = asp.tile([128, 257], f32, tag="sps")
                                nc.tensor.matmul(sp, kt[po:po + 64, g, 1 + ci * 128:129 + ci * 128],
                                                 qt[po:po + 64, g, :], start=True, stop=True)
                                nc.vector.tensor_add(ssb2[:, ci], sp, bsb[:, h, ci])
                            nc.scalar.activation(expst, ssb2, EXP)
                            sp2 = as2.tile([1, 257], f32, tag="sp2")
                            nc.tensor.matmul(sp2, kt[po:po + 64, g, 0:1],
                                             qt[po:po + 64, g, :], start=True, stop=True)
                            ex2 = aw.tile([1, 257], bf16, tag="ex2")
                            nc.scalar.activation(ex2, sp2, EXP)
                            ops = aop.tile([65, 257], f32, tag="ops")
                            nc.tensor.matmul(ops, va[:, 0, h], expst[:, 0], start=True, stop=False)
                            nc.tensor.matmul(ops, va[:, 1, h], expst[:, 1], start=False, stop=False)
                            nc.tensor.matmul(ops, va[:1, 2, h], ex2, start=False, stop=True)
                            nc.vector.tensor_copy(oc65[:, h], ops)
                            nc.sync.dma_start(out=zsb[h:h + 1], in_=oc65[64:65, h])
                        rzall = aw.tile([12, 257], bf16, tag="rzall")
                        with nc.allow_low_precision(reason="1/Z in bf16; <=0.4% uniform scale error per query"):
                            nc.vector.reciprocal(rzall, zsb)
                        for h in range(12):
                            g, po = h // 2, (h % 2) * 64
                            bz = azp.tile([64, 257], f32, tag="bz")
                            nc.tensor.matmul(bz, esb[:, h, :], rzall, start=True, stop=True)
                            oe = aw.tile([64, 257], bf16, tag="oe")
                            nc.vector.tensor_mul(oe, oc65[0:64, h], bz)
                            nc.sync.dma_start(out=oT_d[g, po:po + 64, b * N:(b + 1) * N], in_=oe)

                # ---- Phase 5+6+7 fused: proj+res+LN2+FC1+gelu+FC2+res -> out ----
                with tc.tile_pool(name="fa", bufs=2) as fa, \
                     tc.tile_pool(name="fy", bufs=2) as fy, \
                     tc.tile_pool(name="fh", bufs=2) as fh, \
                     tc.tile_pool(name="fsm", bufs=3) as fsm, \
                     tc.tile_pool(name="pp", bufs=2, space="PSUM") as pp, \
                     tc.tile_pool(name="f1p", bufs=3, space="PSUM") as f1p, \
                     tc.tile_pool(name="f2p", bufs=3, space="PSUM") as f2p:
                    for (c0, cs) in COLT:
                        nsub = (cs + 127) // 128
                        ot = fa.tile([128, 6, 512], bf16, tag="fot")
                        nc.sync.dma_start(out=ot[:, :, :cs],
                                          in_=oT_d[:, :, c0:c0 + cs].rearrange("k p t -> p k t"))
                        y2sb = fy.tile([128, 4, 768], f32, tag="fy2")
                        h2T = fy.tile([128, 6, 512], bf16, tag="fh2T")
                        for s in range(nsub):
                            s0 = c0 + s * 128
                            ss = min(128, c0 + cs - s0)
                            yt = fsm.tile([128, 768], f32, tag="fyt")
                            nc.scalar.dma_start(out=yt[:ss], in_=y_d[s0:s0 + ss, :])
                            y2 = y2sb[:, s]
                            for nh in range(2):
                                ps = pp.tile([128, 384], f32, tag="pps")
                                for k in range(6):
                                    nc.tensor.matmul(ps[:ss], ot[:, k, s * 128:s * 128 + ss],
                                                     wp[:, k, nh * 384:(nh + 1) * 384],
                                                     start=(k == 0), stop=(k == 5))
                                nc.vector.tensor_add(y2[:ss, nh * 384:(nh + 1) * 384], ps[:ss],
                                                     yt[:ss, nh * 384:(nh + 1) * 384])
                            nc.vector.tensor_add(y2[:ss], y2[:ss], pbc[:ss])
                            mv, rs = _ln_pair(tc, nc, fsm, y2, ss)
                            hb = fsm.tile([128, 768], bf16, tag="fph2")
                            nc.vector.tensor_scalar(out=hb[:ss], in0=y2[:ss], scalar1=mv[:ss, 0:1],
                                                    scalar2=rs[:ss], op0=SUB, op1=MUL)
                            sx = max(ss, 16)   # XBAR needs >=16 src rows; extra cols never read
                            for k in range(6):
                                nc.sync.dma_start(out=h2T[:, k, s * 128:s * 128 + sx],
                                                  in_=hb[:sx, k * 128:(k + 1) * 128],
                                                  transpose=True)
                        h3 = fh.tile([128, 24, 512], bf16, tag="fh3")
                        for dch in range(24):
                            ps = f1p.tile([128, 512], f32, tag="f1ps")
                            for k in range(6):
                                nc.tensor.matmul(ps[:, :cs], w1[:, k, dch * 128:(dch + 1) * 128],
                                                 h2T[:, k, :cs], start=(k == 0), stop=(k == 5))
                            nc.scalar.activation(h3[:, dch, :cs], ps[:, :cs],
                                                 mybir.ActivationFunctionType.Gelu,
                                                 bias=b1[:, dch:dch + 1])
                        for s in range(nsub):
                            s0 = c0 + s * 128
                            ss = min(128, c0 + cs - s0)
                            otile = fsm.tile([128, 768], f32, tag="fout")
                            for nh in range(2):
                                ps2 = f2p.tile([128, 384], f32, tag="f2ps")
                                for g in range(24):
                                    nc.tensor.matmul(ps2[:ss], h3[:, g, s * 128:s * 128 + ss],
                                                     w2[:, g, nh * 384:(nh + 1) * 384],
                                                     start=(g == 0), stop=(g == 23))
                                nc.vector.tensor_add(otile[:ss, nh * 384:(nh + 1) * 384], ps2[:ss],
                                                     y2sb[:ss, s, nh * 384:(nh + 1) * 384])
                            nc.vector.tensor_add(otile[:ss], otile[:ss], b2c[:ss])
                            nc.sync.dma_start(out=out_d[s0:s0 + ss, :], in_=otile[:ss])

    nc.compile()
    return nc


def kernel(x, H, W, geo_bias, extra_token, conv_w, conv_b, pe_g, pe_b,
           n1_g, n1_b, qkv_w, rpb_table, proj_w, proj_b, n2_g, n2_b,
           fc1_w, fc1_b, fc2_w, fc2_b):
    x = np.asarray(x, np.float32)
    f = lambda a: np.asarray(a, np.float32)
    geo_bias, extra_token = f(geo_bias), f(extra_token)
    conv_w, conv_b, pe_g, pe_b = f(conv_w), f(conv_b), f(pe_g), f(pe_b)
    n1_g, n1_b, qkv_w, rpb_table = f(n1_g), f(n1_b), f(qkv_w), f(rpb_table)
    proj_w, proj_b, n2_g, n2_b = f(proj_w), f(proj_b), f(n2_g), f(n2_b)
    fc1_w, fc1_b, fc2_w, fc2_b = f(fc1_w), f(fc1_b), f(fc2_w), f(fc2_b)

    if "nc" not in _CACHE:
        _CACHE["nc"] = build()
    nc = _CACHE["nc"]

    # host-side weight prep (layout only; LN scale folds are exact for g=1,b=0)
    cw = conv_w.transpose(2, 3, 1, 0).reshape(3, 3, 3, 128, 768).reshape(27, 128, 768)
    qkv_wf = qkv_w * n1_g[None, :]
    qkv_wf[:D] *= HD ** -0.5
    qkv_b = qkv_w @ n1_b
    qkv_b[:D] *= HD ** -0.5
    fc1_wf = fc1_w * n2_g[None, :]
    fc1_bf = fc1_b + fc1_w @ n2_b
    bias_full = _rel_bias(rpb_table)         # [12, q, key]
    b4 = np.zeros((12, 2, 128, N), np.float32)
    for ci in range(2):
        b4[:, ci] = bias_full[:, :, 1 + ci * 128:129 + ci * 128].transpose(0, 2, 1)
    e12 = np.zeros((12, 12, 64), np.float32)
    for h in range(12):
        e12[h, h, :] = 1.0

    common = {
        "convw": cw.astype(BF),
        "convb_bc": np.tile(conv_b[None, :], (128, 1)).astype(np.float32),
        "peg_bc": np.tile(pe_g[None, :], (128, 1)).astype(np.float32),
        "geo2": (geo_bias[0, 1:, :] + pe_b[None, :]).reshape(2, 128, 768).astype(np.float32),
        "y0row": (extra_token[0] + geo_bias[0, :1, :]).astype(np.float32),
        "qkw": np.ascontiguousarray(qkv_wf[:2 * D].T).reshape(6, 128, 1536).astype(BF),
        "qkb_t": np.ascontiguousarray(qkv_b[:2 * D].reshape(12, 128).T).astype(np.float32),
        "wv_in": np.ascontiguousarray(qkv_wf[2 * D:].T).reshape(6, 128, 768).astype(BF),
        "vb_bc": np.tile(qkv_b[2 * D:][None, :], (128, 1)).astype(np.float32),
        "biasT4": b4.astype(BF),
        "eye12": e12.astype(BF),
        "projw": proj_w.T.reshape(6, 128, 768).astype(BF),
        "projb_bc": np.tile(proj_b[None, :], (128, 1)).astype(np.float32),
        "fc1w": fc1_wf.T.reshape(6, 128, MLP).astype(BF),
        "fc1b_t": np.ascontiguousarray(fc1_bf.reshape(24, 128).T).astype(np.float32),
        "fc2w": fc2_w.T.reshape(24, 128, 768).astype(BF),
        "fc2b_bc": np.tile(fc2_b[None, :], (128, 1)).astype(np.float32),
    }
    in_maps = []
    for c in range(8):
        xs = x[c * BL:(c + 1) * BL].reshape(BL, 3, 128, 32, 32).astype(BF)
        in_maps.append({"x_in": xs, **common})

    global _LAST_MAPS
    _LAST_MAPS = in_maps
    res = bass_utils.run_bass_kernel_spmd(nc, in_maps, core_ids=list(range(8)))
    out = np.concatenate([r["out_d"].reshape(BL, N, D) for r in res.results], axis=0)
    return out.astype(np.float32)


# revision 21
# speedup vs baseline: 1.0980x; 1.0980x over previous
"""MHSA block (patch-embed conv + relative-pos attention + MLP) on 8 NeuronCores.

Sharding: data-parallel over batch (64 images -> 8 per core), weights
replicated. v2:
 - Phase 1 fuses conv + peLN + geo + LN1 + transpose.
 - QKV phase -> q,k feature-major; v token-major with fused ones-column.
 - Attention: S^T per key-chunk with keys reordered (pixel keys first,
   zero-bias extra token last), one batched exp per head, softmax
   denominator piggybacked as a 65th lhsT column, per-image batched
   reciprocal, 1/Z broadcast via tiny block matmuls.
 - proj+LN2+FC1+FC2 fused over 512-token chunks; y2/h2T/h3 SBUF-resident.
 - DMAs batched; issued on sync+scalar DGE.
"""
import numpy as np
import ml_dtypes
import concourse.bass as bass
import concourse.bacc as bacc
import concourse.tile as tile
from concourse import mybir
from concourse import bass_utils
from concourse.masks import make_identity

BF = ml_dtypes.bfloat16
B, CIN, D, HEADS, HD = 64, 384, 768, 12, 64
GS, ET, N = 16, 1, 257
BL = B // 8              # images per core
NT = BL * N              # 2056 packed tokens per core
MLP = 4 * D
COLT = [(c, min(512, NT - c)) for c in range(0, NT, 512)]

_CACHE = {}
_LAST_MAPS = None


def _rel_bias(rpb_table):
    coords = np.stack(np.meshgrid(np.arange(GS), np.arange(GS), indexing='ij'))
    cf = coords.reshape(2, -1)
    rel = (cf[:, :, None] - cf[:, None, :]).transpose(1, 2, 0)
    rel[:, :, 0] += GS - 1
    rel[:, :, 1] += GS - 1
    rel[:, :, 0] *= 2 * GS - 1
    idx = rel.sum(-1)
    out = np.zeros((N, N), dtype=np.int32)
    out[ET:, ET:] = idx
    bias = rpb_table[out]                    # [N, N, HEADS]
    return bias.transpose(2, 0, 1).astype(np.float32)   # [HEADS, q=N, key=N]


def _ln_pair(tc, nc, pools, xt, ts):
    """mean/rstd of xt[:ts, :768] -> (mean, rstd) [ts,1] f32 tiles."""
    st = pools.tile([128, 3, nc.vector.BN_STATS_DIM], mybir.dt.float32, tag="lnst")
    xg = xt.rearrange("p (n f) -> p n f", f=256)
    for i in range(3):
        nc.vector.bn_stats(out=st[:ts, i], in_=xg[:ts, i])
    mv = pools.tile([128, nc.vector.BN_AGGR_DIM], mybir.dt.float32, tag="lnmv")
    nc.vector.bn_aggr(out=mv[:ts], in_=st[:ts])
    eps = pools.tile([128, 1], mybir.dt.float32, tag="lneps")
    nc.vector.memset(eps, 1e-5)
    rs = pools.tile([128, 1], mybir.dt.float32, tag="lnrs")
    nc.scalar.activation(out=rs[:ts], in_=mv[:ts, 1:2],
                         func=mybir.ActivationFunctionType.Sqrt, bias=eps[:ts])
    nc.vector.reciprocal(out=rs[:ts], in_=rs[:ts])
    return mv, rs


def build():
    nc = bacc.Bacc("TRN2", target_bir_lowering=False, debug=False)
    f32, bf16 = mybir.dt.float32, mybir.dt.bfloat16
    SUB, MUL = mybir.AluOpType.subtract, mybir.AluOpType.mult
    EXP = mybir.ActivationFunctionType.Exp
    di = lambda n, s, d: nc.dram_tensor(n, s, d, kind="ExternalInput").ap()
    x_in = di("x_in", [BL, 3, 128, 32, 32], bf16)
    convw = di("convw", [27, 128, 768], bf16)
    convb_bc = di("convb_bc", [128, 768], f32)
    peg_bc = di("peg_bc", [128, 768], f32)
    geo2 = di("geo2", [2, 128, 768], f32)
    y0row = di("y0row", [1, 768], f32)
    qkw = di("qkw", [6, 128, 1536], bf16)
    qkb_t = di("qkb_t", [128, 12], f32)
    wv_in = di("wv_in", [6, 128, 768], bf16)
    vb_bc = di("vb_bc", [128, 768], f32)
    biasT4 = di("biasT4", [12, 2, 128, 257], bf16)
    eye12 = di("eye12", [12, 12, 64], bf16)
    projw = di("projw", [6, 128, 768], bf16)
    projb_bc = di("projb_bc", [128, 768], f32)
    fc1w = di("fc1w", [6, 128, MLP], bf16)
    fc1b_t = di("fc1b_t", [128, 24], f32)
    fc2w = di("fc2w", [24, 128, 768], bf16)
    fc2b_bc = di("fc2b_bc", [128, 768], f32)
    out_d = nc.dram_tensor("out_d", [NT, 768], f32, kind="ExternalOutput").ap()

    with tile.TileContext(nc) as tc:
        with tc.tile_pool(name="dram", bufs=1, space="DRAM") as dpool:
            y_d = dpool.tile([NT, 768], f32)
            h_d = dpool.tile([NT, 768], bf16)
            qkT_d = dpool.tile([12, 128, NT], bf16)
            va_d = dpool.tile([NT, 12, 65], bf16)
            oT_d = dpool.tile([6, 128, NT], bf16)

            # ---- Phase 1: conv + peLN + geo -> y_d; LN1 + transpose -> hT_d ----
            with tc.tile_pool(name="cw", bufs=1) as cw, \
                 tc.tile_pool(name="cx", bufs=2) as cx, \
                 tc.tile_pool(name="cps", bufs=6, space="PSUM") as cps, \
                 tc.tile_pool(name="cy", bufs=3) as cy:
                wsb = cw.tile([128, 27, 768], bf16)
                nc.sync.dma_start(out=wsb, in_=convw.rearrange("i p f -> p i f"))
                cbc = cw.tile([128, 768], f32)
                nc.sync.dma_start(out=cbc, in_=convb_bc)
                pgc = cw.tile([128, 768], f32)
                nc.sync.dma_start(out=pgc, in_=peg_bc)
                gsb = cw.tile([128, 2, 768], f32)
                nc.sync.dma_start(out=gsb, in_=geo2.rearrange("t p f -> p t f"))
                y0sb = cw.tile([1, 768], f32)
                nc.sync.dma_start(out=y0sb, in_=y0row)
                # token-0 row (identical for every image): y store + LN1 + h_d rows
                for b in range(BL):
                    nc.scalar.dma_start(out=y_d[b * N:b * N + 1, :], in_=y0sb)
                mv0, rs0 = _ln_pair(tc, nc, cy, y0sb, 1)
                hb0 = cy.tile([1, 768], bf16, tag="hb0")
                nc.vector.tensor_scalar(out=hb0, in0=y0sb, scalar1=mv0[:1, 0:1],
                                        scalar2=rs0[:1], op0=SUB, op1=MUL)
                for b in range(BL):
                    nc.scalar.dma_start(out=h_d[b * N:b * N + 1, :], in_=hb0)
                for b in range(BL):
                    xp = cx.tile([128, 3, 1089], bf16, tag="xpad")
                    nc.gpsimd.memset(xp, 0.0)
                    for c in range(3):
                        dst = bass.AP(tensor=xp.tensor, offset=xp.offset + c * 1089 + 34,
                                      ap=[xp.ap[0], [33, 32], [1, 32]])
                        nc.sync.dma_start(out=dst, in_=x_in[b, c])
                    for t in range(2):
                        col = cx.tile([128, 27, 128], bf16, tag="col")
                        for kh in range(3):
                            for kw in range(3):
                                idx = (kh * 3 + kw) * 3
                                src = bass.AP(
                                    tensor=xp.tensor,
                                    offset=xp.offset + (16 * t + kh) * 33 + kw,
                                    ap=[xp.ap[0], [1089, 3], [66, 8], [2, 16]])
                                nc.vector.tensor_copy(
                                    col[:, idx:idx + 3].rearrange("p c (r w) -> p c r w", r=8),
                                    src)
                        yt = cy.tile([128, 768], f32, tag="yt")
                        for nh in range(2):
                            ps = cps.tile([128, 384], f32, tag="cpsum")
                            for i in range(27):
                                nc.tensor.matmul(ps, col[:, i], wsb[:, i, nh * 384:(nh + 1) * 384],
                                                 start=(i == 0), stop=(i == 26))
                            nc.vector.tensor_add(yt[:, nh * 384:(nh + 1) * 384], ps,
                                                 cbc[:, nh * 384:(nh + 1) * 384])
                        mv, rs = _ln_pair(tc, nc, cy, yt, 128)
                        nc.vector.tensor_scalar(out=yt, in0=yt, scalar1=mv[:, 0:1], scalar2=rs,
                                                op0=SUB, op1=MUL)
                        nc.vector.tensor_mul(yt, yt, pgc)
                        nc.vector.tensor_add(yt, yt, gsb[:, t])
                        r0 = b * N + 1 + t * 128
                        nc.scalar.dma_start(out=y_d[r0:r0 + 128, :], in_=yt)
                        mv1, rs1 = _ln_pair(tc, nc, cy, yt, 128)
                        hb = cy.tile([128, 768], bf16, tag="hb")
                        nc.vector.tensor_scalar(out=hb, in0=yt, scalar1=mv1[:, 0:1],
                                                scalar2=rs1, op0=SUB, op1=MUL)
                        nc.sync.dma_start(out=h_d[r0:r0 + 128, :], in_=hb)

            # ---- Phase 3: QKV -> qkT_d (q,k feature-major), va_d (v rows + ones) ----
            with tc.tile_pool(name="qw", bufs=1) as qw, \
                 tc.tile_pool(name="qa", bufs=2) as qa, \
                 tc.tile_pool(name="qs", bufs=2) as qs_pool, \
                 tc.tile_pool(name="qp", bufs=4, space="PSUM") as qp, \
                 tc.tile_pool(name="vp", bufs=4, space="PSUM") as vp:
                wq = qw.tile([128, 6, 1536], bf16)
                nc.sync.dma_start(out=wq, in_=qkw.rearrange("k p f -> p k f"))
                wvs = qw.tile([128, 6, 768], bf16)
                nc.sync.dma_start(out=wvs, in_=wv_in.rearrange("k p f -> p k f"))
                qb = qw.tile([128, 12], f32)
                nc.sync.dma_start(out=qb, in_=qkb_t)
                vbc = qw.tile([128, 768], f32)
                nc.sync.dma_start(out=vbc, in_=vb_bc)
                for (c0, cs) in COLT:
                    ht = qa.tile([128, 6, 512], bf16, tag="qh")
                    for k in range(6):
                        src = h_d[c0:c0 + cs, k * 128:(k + 1) * 128]
                        if cs % 16 == 0:
                            nc.sync.dma_start(out=ht[:, k, :cs], in_=src, transpose=True)
                        else:
                            nc.sync.dma_start(out=ht[:, k, :cs], in_=src.rearrange("a b -> b a"))
                    ev = qs_pool.tile([128, 12, 512], bf16, tag="qev")
                    for dch in range(12):
                        ps = qp.tile([128, 512], f32, tag="qps")
                        for k in range(6):
                            nc.tensor.matmul(ps[:, :cs], wq[:, k, dch * 128:(dch + 1) * 128],
                                             ht[:, k, :cs], start=(k == 0), stop=(k == 5))
                        nc.vector.tensor_scalar_add(out=ev[:, dch, :cs], in0=ps[:, :cs],
                                                    scalar1=qb[:, dch:dch + 1])
                    nc.sync.dma_start(out=qkT_d[:, :, c0:c0 + cs].rearrange("d p t -> p d t"),
                                      in_=ev[:, :, :cs])
                    nsub = (cs + 127) // 128
                    for s in range(nsub):
                        s0 = c0 + s * 128
                        ss = min(128, c0 + cs - s0)
                        vsb = qs_pool.tile([128, 12, 65], bf16, tag="vsb")
                        nc.gpsimd.memset(vsb[:ss], 1.0)
                        for half in range(2):
                            psv = vp.tile([128, 384], f32, tag="vps")
                            for k in range(6):
                                nc.tensor.matmul(psv[:ss], ht[:, k, s * 128:s * 128 + ss],
                                                 wvs[:, k, half * 384:(half + 1) * 384],
                                                 start=(k == 0), stop=(k == 5))
                            nc.vector.tensor_add(
                                vsb[:ss, half * 6:(half + 1) * 6, 0:64],
                                psv[:ss].rearrange("p (h f) -> p h f", f=64),
                                vbc[:ss, half * 384:(half + 1) * 384].rearrange("p (h f) -> p h f", f=64))
                        nc.scalar.dma_start(out=va_d[s0:s0 + ss], in_=vsb[:ss])

            # ---- late weights (prefetch during attention) ----
            with tc.tile_pool(name="fw", bufs=1) as fw:
                wp = fw.tile([128, 6, 768], bf16)
                nc.sync.dma_start(out=wp, in_=projw.rearrange("k p f -> p k f"))
                pbc = fw.tile([128, 768], f32)
                nc.sync.dma_start(out=pbc, in_=projb_bc)
                w1 = fw.tile([128, 6, MLP], bf16)
                nc.sync.dma_start(out=w1, in_=fc1w.rearrange("k p f -> p k f"))
                b1 = fw.tile([128, 24], f32)
                nc.sync.dma_start(out=b1, in_=fc1b_t)
                w2 = fw.tile([128, 24, 768], bf16)
                nc.sync.dma_start(out=w2, in_=fc2w.rearrange("k p f -> p k f"))
                b2c = fw.tile([128, 768], f32)
                nc.sync.dma_start(out=b2c, in_=fc2b_bc)
                idb2 = fw.tile([128, 128], bf16)
                make_identity(nc, idb2)

                # ---- Phase 4: attention -> oT_d ----
                with tc.tile_pool(name="ac", bufs=1) as ac, \
                     tc.tile_pool(name="aq", bufs=2) as aq, \
                     tc.tile_pool(name="aw", bufs=4) as aw, \
                     tc.tile_pool(name="asp", bufs=2, space="PSUM") as asp, \
                     tc.tile_pool(name="as2", bufs=2, space="PSUM") as as2, \
                     tc.tile_pool(name="aop", bufs=2, space="PSUM") as aop, \
                     tc.tile_pool(name="azp", bufs=2, space="PSUM") as azp:
                    bsb = ac.tile([128, 12, 2, 257], bf16)
                    nc.sync.dma_start(out=bsb, in_=biasT4.rearrange("h c p q -> p h c q"))
                    esb = ac.tile([12, 12, 64], bf16)
                    nc.sync.dma_start(out=esb, in_=eye12)
                    for b in range(BL):
                        qt = aq.tile([128, 6, 257], bf16, tag="qt")
                        nc.sync.dma_start(out=qt, in_=qkT_d[0:6, :, b * N:(b + 1) * N].rearrange("g p t -> p g t"))
                        kt = aq.tile([128, 6, 257], bf16, tag="kt")
                        nc.sync.dma_start(out=kt, in_=qkT_d[6:12, :, b * N:(b + 1) * N].rearrange("g p t -> p g t"))
                        va = aq.tile([128, 3, 12, 65], bf16, tag="va")
                        nc.scalar.dma_start(out=va[:, 0], in_=va_d[b * N + 1:b * N + 129])
                        nc.scalar.dma_start(out=va[:, 1], in_=va_d[b * N + 129:b * N + 257])
                        nc.scalar.dma_start(out=va[:1, 2], in_=va_d[b * N:b * N + 1])
                        oc65 = aq.tile([65, 12, 257], f32, tag="oc65")
                        zsb = aq.tile([12, 257], f32, tag="zsb")
                        for h in range(12):
                            g, po = h // 2, (h % 2) * 64
                            ssb2 = aw.tile([128, 2, 257], bf16, tag="ssb2")
                            expst = aw.tile([128, 2, 257], bf16, tag="expst")
                            for ci in range(2):
                                sp = asp.tile([128, 257], f32, tag="sps")
                                nc.tensor.matmul(sp, kt[po:po + 64, g, 1 + ci * 128:129 + ci * 128],
                                                 qt[po:po + 64, g, :], start=True, stop=True)
                                nc.vector.tensor_add(ssb2[:, ci], sp, bsb[:, h, ci])
                            nc.scalar.activation(expst, ssb2, EXP)
                            sp2 = as2.tile([1, 257], f32, tag="sp2")
                            nc.tensor.matmul(sp2, kt[po:po + 64, g, 0:1],
                                             qt[po:po + 64, g, :], start=True, stop=True)
                            ex2 = aw.tile([1, 257], bf16, tag="ex2")
                            nc.scalar.activation(ex2, sp2, EXP)
                            ops = aop.tile([65, 257], f32, tag="ops")
                            nc.tensor.matmul(ops, va[:, 0, h], expst[:, 0], start=True, stop=False)
                            nc.tensor.matmul(ops, va[:, 1, h], expst[:, 1], start=False, stop=False)
                            nc.tensor.matmul(ops, va[:1, 2, h], ex2, start=False, stop=True)
                            nc.vector.tensor_copy(oc65[:, h], ops)
                            nc.sync.dma_start(out=zsb[h:h + 1], in_=oc65[64:65, h])
                        rzall = aw.tile([12, 257], bf16, tag="rzall")
                        with nc.allow_low_precision(reason="1/Z in bf16; <=0.4% uniform scale error per query"):
                            nc.vector.reciprocal(rzall, zsb)
                        for h in range(12):
                            g, po = h // 2, (h % 2) * 64
                            bz = azp.tile([64, 257], f32, tag="bz")
                            nc.tensor.matmul(bz, esb[:, h, :], rzall, start=True, stop=True)
                            oe = aw.tile([64, 257], bf16, tag="oe")
                            nc.vector.tensor_mul(oe, oc65[0:64, h], bz)
                            nc.sync.dma_start(out=oT_d[g, po:po + 64, b * N:(b + 1) * N], in_=oe)

                # ---- Phase 5+6+7 fused: proj+res+LN2+FC1+gelu+FC2+res -> out ----
                with tc.tile_pool(name="fa", bufs=2) as fa, \
                     tc.tile_pool(name="fy", bufs=2) as fy, \
                     tc.tile_pool(name="fh", bufs=2) as fh, \
                     tc.tile_pool(name="fsm", bufs=3) as fsm, \
                     tc.tile_pool(name="pp", bufs=2, space="PSUM") as pp, \
                     tc.tile_pool(name="ptp", bufs=2, space="PSUM") as ptp, \
                     tc.tile_pool(name="f1p", bufs=2, space="PSUM") as f1p, \
                     tc.tile_pool(name="f2p", bufs=2, space="PSUM") as f2p:
                    for (c0, cs) in COLT:
                        nsub = (cs + 127) // 128
                        ot = fa.tile([128, 6, 512], bf16, tag="fot")
                        nc.sync.dma_start(out=ot[:, :, :cs],
                                          in_=oT_d[:, :, c0:c0 + cs].rearrange("k p t -> p k t"))
                        y2sb = fy.tile([128, 4, 768], f32, tag="fy2")
                        h2T = fy.tile([128, 6, 512], bf16, tag="fh2T")
                        for s in range(nsub):
                            s0 = c0 + s * 128
                            ss = min(128, c0 + cs - s0)
                            yt = fsm.tile([128, 768], f32, tag="fyt")
                            nc.scalar.dma_start(out=yt[:ss], in_=y_d[s0:s0 + ss, :])
                            y2 = y2sb[:, s]
                            for nh in range(2):
                                ps = pp.tile([128, 384], f32, tag="pps")
                                for k in range(6):
                                    nc.tensor.matmul(ps[:ss], ot[:, k, s * 128:s * 128 + ss],
                                                     wp[:, k, nh * 384:(nh + 1) * 384],
                                                     start=(k == 0), stop=(k == 5))
                                nc.vector.tensor_add(y2[:ss, nh * 384:(nh + 1) * 384], ps[:ss],
                                                     yt[:ss, nh * 384:(nh + 1) * 384])
                            nc.vector.tensor_add(y2[:ss], y2[:ss], pbc[:ss])
                            mv, rs = _ln_pair(tc, nc, fsm, y2, ss)
                            hb = fsm.tile([128, 768], bf16, tag="fph2")
                            nc.vector.tensor_scalar(out=hb[:ss], in0=y2[:ss], scalar1=mv[:ss, 0:1],
                                                    scalar2=rs[:ss], op0=SUB, op1=MUL)
                            for k in range(6):
                                tp = ptp.tile([128, 128], bf16, tag="fptr")
                                nc.tensor.transpose(tp[:, :ss], hb[:ss, k * 128:(k + 1) * 128],
                                                    idb2[:ss, :ss])
                                nc.vector.tensor_copy(h2T[:, k, s * 128:s * 128 + ss], tp[:, :ss])
                        h3 = fh.tile([128, 24, 512], bf16, tag="fh3")
                        for dch in range(24):
                            ps = f1p.tile([128, 512], f32, tag="f1ps")
                            for k in range(6):
                                nc.tensor.matmul(ps[:, :cs], w1[:, k, dch * 128:(dch + 1) * 128],
                                                 h2T[:, k, :cs], start=(k == 0), stop=(k == 5))
                            nc.scalar.activation(h3[:, dch, :cs], ps[:, :cs],
                                                 mybir.ActivationFunctionType.Gelu,
                                                 bias=b1[:, dch:dch + 1])
                        for s in range(nsub):
                            s0 = c0 + s * 128
                            ss = min(128, c0 + cs - s0)
                            otile = fsm.tile([128, 768], f32, tag="fout")
                            for nh in range(2):
                                ps2 = f2p.tile([128, 384], f32, tag="f2ps")
                                for g in range(24):
                                    nc.tensor.matmul(ps2[:ss], h3[:, g, s * 128:s * 128 + ss],
                                                     w2[:, g, nh * 384:(nh + 1) * 384],
                                                     start=(g == 0), stop=(g == 23))
                                nc.vector.tensor_add(otile[:ss, nh * 384:(nh + 1) * 384], ps2[:ss],
                                                     y2sb[:ss, s, nh * 384:(nh + 1) * 384])
                            nc.vector.tensor_add(otile[:ss], otile[:ss], b2c[:ss])
                            nc.sync.dma_start(out=out_d[s0:s0 + ss, :], in_=otile[:ss])

    nc.compile()
    return nc


def kernel(x, H, W, geo_bias, extra_token, conv_w, conv_b, pe_g, pe_b,
           n1_g, n1_b, qkv_w, rpb_table, proj_w, proj_b, n2_g, n2_b,
           fc1_w, fc1_b, fc2_w, fc2_b):
    x = np.asarray(x, np.float32)
    f = lambda a: np.asarray(a, np.float32)
    geo_bias, extra_token = f(geo_bias), f(extra_token)
    conv_w, conv_b, pe_g, pe_b = f(conv_w), f(conv_b), f(pe_g), f(pe_b)
    n1_g, n1_b, qkv_w, rpb_table = f(n1_g), f(n1_b), f(qkv_w), f(rpb_table)
    proj_w, proj_b, n2_g, n2_b = f(proj_w), f(proj_b), f(n2_g), f(n2_b)
    fc1_w, fc1_b, fc2_w, fc2_b = f(fc1_w), f(fc1_b), f(fc2_w), f(fc2_b)

    if "nc" not in _CACHE:
        _CACHE["nc"] = build()
    nc = _CACHE["nc"]

    # host-side weight prep (layout only; LN scale folds are exact for g=1,b=0)
    cw = conv_w.transpose(2, 3, 1, 0).reshape(3, 3, 3, 128, 768).reshape(27, 128, 768)
    qkv_wf = qkv_w * n1_g[None, :]
    qkv_wf[:D] *= HD ** -0.5
    qkv_b = qkv_w @ n1_b
    qkv_b[:D] *= HD ** -0.5
    fc1_wf = fc1_w * n2_g[None, :]
    fc1_bf = fc1_b + fc1_w @ n2_b
    bias_full = _rel_bias(rpb_table)         # [12, q, key]
    b4 = np.zeros((12, 2, 128, N), np.float32)
    for ci in range(2):
        b4[:, ci] = bias_full[:, :, 1 + ci * 128:129 + ci * 128].transpose(0, 2, 1)
    e12 = np.zeros((12, 12, 64), np.float32)
    for h in range(12):
        e12[h, h, :] = 1.0

    common = {
        "convw": cw.astype(BF),
        "convb_bc": np.tile(conv_b[None, :], (128, 1)).astype(np.float32),
        "peg_bc": np.tile(pe_g[None, :], (128, 1)).astype(np.float32),
        "geo2": (geo_bias[0, 1:, :] + pe_b[None, :]).reshape(2, 128, 768).astype(np.float32),
        "y0row": (extra_token[0] + geo_bias[0, :1, :]).astype(np.float32),
        "qkw": np.ascontiguousarray(qkv_wf[:2 * D].T).reshape(6, 128, 1536).astype(BF),
        "qkb_t": np.ascontiguousarray(qkv_b[:2 * D].reshape(12, 128).T).astype(np.float32),
        "wv_in": np.ascontiguousarray(qkv_wf[2 * D:].T).reshape(6, 128, 768).astype(BF),
        "vb_bc": np.tile(qkv_b[2 * D:][None, :], (128, 1)).astype(np.float32),
        "biasT4": b4.astype(BF),
        "eye12": e12.astype(BF),
        "projw": proj_w.T.reshape(6, 128, 768).astype(BF),
        "projb_bc": np.tile(proj_b[None, :], (128, 1)).astype(np.float32),
        "fc1w": fc1_wf.T.reshape(6, 128, MLP).astype(BF),
        "fc1b_t": np.ascontiguousarray(fc1_bf.reshape(24, 128).T).astype(np.float32),
        "fc2w": fc2_w.T.reshape(24, 128, 768).astype(BF),
        "fc2b_bc": np.tile(fc2_b[None, :], (128, 1)).astype(np.float32),
    }
    in_maps = []
    for c in range(8):
        xs = x[c * BL:(c + 1) * BL].reshape(BL, 3, 128, 32, 32).astype(BF)
        in_maps.append({"x_in": xs, **common})

    global _LAST_MAPS
    _LAST_MAPS = in_maps
    res = bass_utils.run_bass_kernel_spmd(nc, in_maps, core_ids=list(range(8)))
    out = np.concatenate([r["out_d"].reshape(BL, N, D) for r in res.results], axis=0)
    return out.astype(np.float32)
